# revision 17
# baseline (speedup 1.0000x reference)
"""Balanced-topk masking kernel for Trainium2 (8 NeuronCores, SPMD data-parallel).

Reference semantics (per (token, bank) row of 128 elements):
    scores = |x| + bias          (bias is zeros in the graded input)
    keep the top-16 scores (ties broken by lowest index, like jax.lax.top_k),
    zero the rest; out = x * mask.  num_assigned_tokens passes through.

Device algorithm per row (exact, tie-correct for any input):
    s  = |x|  (+ bias on the general path)        [Act engine]
    m1 = max8(s);  match_replace(m1 in s -> SENTINEL)   # removes top-8,
    m2 = max8(s);  match_replace(m2 in s -> SENTINEL)   # then ranks 9-16.
    match_replace consumes one occurrence per matched value, scanning from
    index 0 -- identical multiplicity/tie semantics to top_k.  [DVE engine]
    fast path:  mask = relu(-s) (1 at sentinel spots)   [Act]
                out  = mask * x                         [Pool]
    bias path:  out  = (s == SENTINEL) * x  fused       [DVE]

Sharding: x [4,4096,4096] -> tokens [16384, 4096]; 2048 contiguous tokens per
core; bias replicated (only loaded when nonzero).
"""

import sys

for _p in ("/opt/trn_rl_repo", "/opt/pypackages"):
    if _p not in sys.path:
        sys.path.insert(0, _p)

import numpy as np

import concourse.bacc as bacc
import concourse.bass as bass
import concourse.mybir as mybir
import concourse.tile as tile
from concourse.bass_utils import run_bass_kernel_spmd

N_CORES = 8
B, S, H = 4, 4096, 4096
TOKENS = B * S                      # 16384
TOK_PER_CORE = TOKENS // N_CORES    # 2048
BANK = 128
N_BANKS = H // BANK                 # 32
TOPK = 16
ROWS_PER_CORE = TOK_PER_CORE * N_BANKS  # 65536

K_ROWS = 32                         # (token,bank) rows per partition per tile
TILE_F = K_ROWS * BANK              # 4096 fp32 elements of free dim
TILE_ROWS = 128 * K_ROWS            # 4096 rows per tile
N_TILES = ROWS_PER_CORE // TILE_ROWS  # 16

F32 = mybir.dt.float32
FAST_SENTINEL = -1.0                # scores >= 0 on the fast (bias==0) path
SLOW_SENTINEL = -float(2.0 ** 126)  # general path sentinel, matched by equality

_program_cache: dict = {}


def _build_program(with_bias: bool, n_tiles: int = N_TILES) -> bass.Bass:
    nc = bacc.Bacc(
        "TRN2",
        target_bir_lowering=False,
        debug=False,
        enable_asserts=False,
        num_devices=N_CORES,
    )
    x = nc.dram_tensor("x", [TOK_PER_CORE, H], F32, kind="ExternalInput")
    out = nc.dram_tensor("out", [TOK_PER_CORE, H], F32, kind="ExternalOutput")
    bias = (
        nc.dram_tensor("bias", [N_BANKS, BANK], F32, kind="ExternalInput")
        if with_bias
        else None
    )
    sentinel = SLOW_SENTINEL if with_bias else FAST_SENTINEL

    with tile.TileContext(nc) as tc:
        with (
            tc.tile_pool(name="xp", bufs=3) as xp,
            tc.tile_pool(name="sp", bufs=2) as sp,
            tc.tile_pool(name="mp", bufs=6) as mp,
            tc.tile_pool(name="kp", bufs=2) as kp,
            tc.tile_pool(name="op", bufs=2) as op,
            tc.tile_pool(name="bp", bufs=1) as bp,
        ):
            bt = None
            if with_bias:
                # with K_ROWS == 32, row (t, p, k) has bank == k, so the bias
                # tile is partition-independent: bt[p, k*128+e] = bias[k, e]
                bt = bp.tile([128, TILE_F], F32)
                nc.sync.dma_start(
                    bt[:],
                    bass.AP(bias, 0, [[0, 128], [BANK, K_ROWS], [1, BANK]]),
                )

            # Software-pipelined prologue: load + abs of tile t+1 are issued
            # BEFORE tile t's mask/select work, so the Act engine never makes
            # the DVE wait at tile boundaries.
            def load_and_abs(t):
                base = t * 128 * TILE_F
                xt = xp.tile([128, TILE_F], F32)
                nc.sync.dma_start(
                    xt[:], bass.AP(x, base, [[TILE_F, 128], [1, TILE_F]])
                )
                # s = |x| on Act (sign-bit clear; verified bit-exact on HW)
                st = sp.tile([128, TILE_F], F32)
                nc.scalar.activation(
                    st[:], xt[:], mybir.ActivationFunctionType.Abs
                )
                if with_bias:
                    nc.gpsimd.tensor_tensor(
                        st[:], st[:], bt[:], op=mybir.AluOpType.add
                    )
                return xt, st

            pending = load_and_abs(0)
            for t in range(n_tiles):
                base = t * 128 * TILE_F
                xt, st = pending

                m1 = mp.tile([128, 8 * K_ROWS], F32)
                m2 = mp.tile([128, 8 * K_ROWS], F32)
                for k in range(K_ROWS):
                    nc.vector.max(
                        out=m1[:, k * 8 : (k + 1) * 8],
                        in_=st[:, k * BANK : (k + 1) * BANK],
                    )
                for k in range(K_ROWS):
                    sl = st[:, k * BANK : (k + 1) * BANK]
                    nc.vector.match_replace(
                        out=sl,
                        in_to_replace=m1[:, k * 8 : (k + 1) * 8],
                        in_values=sl,
                        imm_value=sentinel,
                    )
                for k in range(K_ROWS):
                    nc.vector.max(
                        out=m2[:, k * 8 : (k + 1) * 8],
                        in_=st[:, k * BANK : (k + 1) * BANK],
                    )
                for k in range(K_ROWS):
                    sl = st[:, k * BANK : (k + 1) * BANK]
                    nc.vector.match_replace(
                        out=sl,
                        in_to_replace=m2[:, k * 8 : (k + 1) * 8],
                        in_values=sl,
                        imm_value=sentinel,
                    )

                if t + 1 < n_tiles:
                    pending = load_and_abs(t + 1)

                ot = op.tile([128, TILE_F], F32)
                if with_bias:
                    # general path: select by exact sentinel equality on DVE
                    nc.vector.scalar_tensor_tensor(
                        ot[:],
                        st[:],
                        sentinel,
                        xt[:],
                        op0=mybir.AluOpType.is_equal,
                        op1=mybir.AluOpType.mult,
                    )
                else:
                    # fast path: scores >= 0, sentinel -1.  mask = relu(-s)
                    # is exactly 1.0 at the 16 sentinel positions, 0 elsewhere
                    mt = kp.tile([128, TILE_F], F32)
                    nc.scalar.activation(
                        mt[:],
                        st[:],
                        mybir.ActivationFunctionType.Relu,
                        scale=-1.0,
                    )
                    nc.gpsimd.tensor_tensor(
                        ot[:], mt[:], xt[:], op=mybir.AluOpType.mult
                    )

                nc.scalar.dma_start(
                    bass.AP(out, base, [[TILE_F, 128], [1, TILE_F]]), ot[:]
                )
    nc.compile()
    return nc


def _get_program(with_bias: bool) -> bass.Bass:
    key = ("v3", with_bias)
    if key not in _program_cache:
        _program_cache[key] = _build_program(with_bias)
    return _program_cache[key]


def _run_on_hw(x_flat: np.ndarray, bias_np: np.ndarray, trace: bool = False):
    """x_flat: [TOKENS, H] float32. Returns (out_flat [TOKENS, H], results)."""
    with_bias = bool(np.any(bias_np))
    nc = _get_program(with_bias)
    shards = x_flat.reshape(N_CORES, TOK_PER_CORE, H)
    in_maps = []
    for c in range(N_CORES):
        m = {"x": np.ascontiguousarray(shards[c])}
        if with_bias:
            m["bias"] = np.ascontiguousarray(bias_np)
        in_maps.append(m)
    res = run_bass_kernel_spmd(
        nc, in_maps, core_ids=list(range(N_CORES)), trace=trace
    )
    out_flat = np.concatenate(
        [r["out"].reshape(1, TOK_PER_CORE, H) for r in res.results], axis=0
    ).reshape(TOKENS, H)
    return out_flat, res


def kernel(x, bias, num_assigned_tokens):
    x_np = np.ascontiguousarray(np.asarray(x, dtype=np.float32))
    bias_np = np.asarray(bias, dtype=np.float32)
    nat = np.asarray(num_assigned_tokens, dtype=np.float32)
    assert x_np.shape == (B, S, H), x_np.shape

    out_flat, _ = _run_on_hw(x_np.reshape(TOKENS, H), bias_np)
    return out_flat.reshape(B, S, H), nat


# revision 20
# speedup vs baseline: 1.0319x; 1.0319x over previous
"""Balanced-topk masking kernel for Trainium2 (8 NeuronCores, SPMD data-parallel).

Reference semantics (per (token, bank) row of 128 elements):
    scores = |x| + bias          (bias is zeros in the graded input)
    keep the top-16 scores (ties broken by lowest index, like jax.lax.top_k),
    zero the rest; out = x * mask.  num_assigned_tokens passes through.

Device algorithm per row (exact, tie-correct for any input):
    s  = |x|  (+ bias on the general path)        [Act engine]
    m1 = max8(s);  match_replace(m1 in s -> SENTINEL)   # removes top-8,
    m2 = max8(s);  match_replace(m2 in s -> SENTINEL)   # then ranks 9-16.
    match_replace consumes one occurrence per matched value, scanning from
    index 0 -- identical multiplicity/tie semantics to top_k.  [DVE engine]
    fast path:  mask = relu(-s) (1 at sentinel spots)   [Act]
                out  = mask * x                         [Pool]
    bias path:  out  = (s == SENTINEL) * x  fused       [DVE]

Sharding: x [4,4096,4096] -> tokens [16384, 4096]; 2048 contiguous tokens per
core; bias replicated (only loaded when nonzero).
"""

import sys

for _p in ("/opt/trn_rl_repo", "/opt/pypackages"):
    if _p not in sys.path:
        sys.path.insert(0, _p)

import numpy as np

import concourse.bacc as bacc
import concourse.bass as bass
import concourse.mybir as mybir
import concourse.tile as tile
from concourse.bass_utils import run_bass_kernel_spmd

N_CORES = 8
B, S, H = 4, 4096, 4096
TOKENS = B * S                      # 16384
TOK_PER_CORE = TOKENS // N_CORES    # 2048
BANK = 128
N_BANKS = H // BANK                 # 32
TOPK = 16
ROWS_PER_CORE = TOK_PER_CORE * N_BANKS  # 65536

K_ROWS = 32                         # (token,bank) rows per partition per tile
TILE_F = K_ROWS * BANK              # 4096 fp32 elements of free dim
TILE_ROWS = 128 * K_ROWS            # 4096 rows per tile
N_TILES = ROWS_PER_CORE // TILE_ROWS  # 16

F32 = mybir.dt.float32
FAST_SENTINEL = -1.0                # scores >= 0 on the fast (bias==0) path
SLOW_SENTINEL = -float(2.0 ** 126)  # general path sentinel, matched by equality

_program_cache: dict = {}


def _build_program(with_bias: bool, n_tiles: int = N_TILES) -> bass.Bass:
    nc = bacc.Bacc(
        "TRN2",
        target_bir_lowering=False,
        debug=False,
        enable_asserts=False,
        num_devices=N_CORES,
    )
    x = nc.dram_tensor("x", [TOK_PER_CORE, H], F32, kind="ExternalInput")
    out = nc.dram_tensor("out", [TOK_PER_CORE, H], F32, kind="ExternalOutput")
    bias = (
        nc.dram_tensor("bias", [N_BANKS, BANK], F32, kind="ExternalInput")
        if with_bias
        else None
    )
    sentinel = SLOW_SENTINEL if with_bias else FAST_SENTINEL

    with tile.TileContext(nc) as tc:
        with (
            tc.tile_pool(name="xp", bufs=3) as xp,
            tc.tile_pool(name="sp", bufs=2) as sp,
            tc.tile_pool(name="mp", bufs=6) as mp,
            tc.tile_pool(name="kp", bufs=2) as kp,
            tc.tile_pool(name="op", bufs=2) as op,
            tc.tile_pool(name="bp", bufs=1) as bp,
        ):
            bt = None
            if with_bias:
                # with K_ROWS == 32, row (t, p, k) has bank == k, so the bias
                # tile is partition-independent: bt[p, k*128+e] = bias[k, e]
                bt = bp.tile([128, TILE_F], F32)
                nc.sync.dma_start(
                    bt[:],
                    bass.AP(bias, 0, [[0, 128], [BANK, K_ROWS], [1, BANK]]),
                )

            # Software-pipelined prologue: load + abs of tile t+1 are issued
            # BEFORE tile t's mask/select work, so the Act engine never makes
            # the DVE wait at tile boundaries.  The first tile is loaded in
            # quarters so the DVE can start after ~1/4 of the DMA+abs latency
            # (subtile deps let max8 of slice k wait only on its quarter).
            def load_and_abs(t):
                base = t * 128 * TILE_F
                xt = xp.tile([128, TILE_F], F32)
                st = sp.tile([128, TILE_F], F32)
                n_parts = 4 if t == 0 else 1
                q = TILE_F // n_parts
                for i in range(n_parts):
                    fr = slice(i * q, (i + 1) * q)
                    nc.sync.dma_start(
                        xt[:, fr],
                        bass.AP(x, base + i * q, [[TILE_F, 128], [1, q]]),
                    )
                    # s = |x| on Act (sign-bit clear; verified bit-exact on HW)
                    nc.scalar.activation(
                        st[:, fr], xt[:, fr], mybir.ActivationFunctionType.Abs
                    )
                    if with_bias:
                        nc.gpsimd.tensor_tensor(
                            st[:, fr], st[:, fr], bt[:, fr],
                            op=mybir.AluOpType.add,
                        )
                return xt, st

            pending = load_and_abs(0)
            for t in range(n_tiles):
                base = t * 128 * TILE_F
                xt, st = pending

                m1 = mp.tile([128, 8 * K_ROWS], F32)
                m2 = mp.tile([128, 8 * K_ROWS], F32)
                for k in range(K_ROWS):
                    nc.vector.max(
                        out=m1[:, k * 8 : (k + 1) * 8],
                        in_=st[:, k * BANK : (k + 1) * BANK],
                    )
                for k in range(K_ROWS):
                    sl = st[:, k * BANK : (k + 1) * BANK]
                    nc.vector.match_replace(
                        out=sl,
                        in_to_replace=m1[:, k * 8 : (k + 1) * 8],
                        in_values=sl,
                        imm_value=sentinel,
                    )
                for k in range(K_ROWS):
                    nc.vector.max(
                        out=m2[:, k * 8 : (k + 1) * 8],
                        in_=st[:, k * BANK : (k + 1) * BANK],
                    )
                for k in range(K_ROWS):
                    sl = st[:, k * BANK : (k + 1) * BANK]
                    nc.vector.match_replace(
                        out=sl,
                        in_to_replace=m2[:, k * 8 : (k + 1) * 8],
                        in_values=sl,
                        imm_value=sentinel,
                    )

                if t + 1 < n_tiles:
                    pending = load_and_abs(t + 1)

                # The last tile's mask/multiply/store runs in quarters: each
                # quarter only depends on its own slices' round-2 results, so
                # the post-DVE tail shrinks to ~1/4 of a tile.
                ot = op.tile([128, TILE_F], F32)
                mt = None if with_bias else kp.tile([128, TILE_F], F32)
                n_parts = 4 if t == n_tiles - 1 else 1
                q = TILE_F // n_parts
                for i in range(n_parts):
                    fr = slice(i * q, (i + 1) * q)
                    if with_bias:
                        # general path: select by exact sentinel equality on DVE
                        nc.vector.scalar_tensor_tensor(
                            ot[:, fr],
                            st[:, fr],
                            sentinel,
                            xt[:, fr],
                            op0=mybir.AluOpType.is_equal,
                            op1=mybir.AluOpType.mult,
                        )
                    else:
                        # fast path: scores >= 0, sentinel -1.  mask = relu(-s)
                        # is exactly 1.0 at the sentinel positions, 0 elsewhere
                        nc.scalar.activation(
                            mt[:, fr],
                            st[:, fr],
                            mybir.ActivationFunctionType.Relu,
                            scale=-1.0,
                        )
                        nc.gpsimd.tensor_tensor(
                            ot[:, fr], mt[:, fr], xt[:, fr],
                            op=mybir.AluOpType.mult,
                        )
                    nc.scalar.dma_start(
                        bass.AP(out, base + i * q, [[TILE_F, 128], [1, q]]),
                        ot[:, fr],
                    )
    nc.compile()
    return nc


def _get_program(with_bias: bool) -> bass.Bass:
    key = ("v3", with_bias)
    if key not in _program_cache:
        _program_cache[key] = _build_program(with_bias)
    return _program_cache[key]


def _run_on_hw(x_flat: np.ndarray, bias_np: np.ndarray, trace: bool = False):
    """x_flat: [TOKENS, H] float32. Returns (out_flat [TOKENS, H], results)."""
    with_bias = bool(np.any(bias_np))
    nc = _get_program(with_bias)
    shards = x_flat.reshape(N_CORES, TOK_PER_CORE, H)
    in_maps = []
    for c in range(N_CORES):
        m = {"x": np.ascontiguousarray(shards[c])}
        if with_bias:
            m["bias"] = np.ascontiguousarray(bias_np)
        in_maps.append(m)
    res = run_bass_kernel_spmd(
        nc, in_maps, core_ids=list(range(N_CORES)), trace=trace
    )
    out_flat = np.concatenate(
        [r["out"].reshape(1, TOK_PER_CORE, H) for r in res.results], axis=0
    ).reshape(TOKENS, H)
    return out_flat, res


def kernel(x, bias, num_assigned_tokens):
    x_np = np.ascontiguousarray(np.asarray(x, dtype=np.float32))
    bias_np = np.asarray(bias, dtype=np.float32)
    nat = np.asarray(num_assigned_tokens, dtype=np.float32)
    assert x_np.shape == (B, S, H), x_np.shape

    out_flat, _ = _run_on_hw(x_np.reshape(TOKENS, H), bias_np)
    return out_flat.reshape(B, S, H), nat


# revision 21
# speedup vs baseline: 1.0326x; 1.0006x over previous
"""Balanced-topk masking kernel for Trainium2 (8 NeuronCores, SPMD data-parallel).

Reference semantics (per (token, bank) row of 128 elements):
    scores = |x| + bias          (bias is zeros in the graded input)
    keep the top-16 scores (ties broken by lowest index, like jax.lax.top_k),
    zero the rest; out = x * mask.  num_assigned_tokens passes through.

Device algorithm per row (exact, tie-correct for any input):
    s  = |x|  (+ bias on the general path)        [Act engine]
    m1 = max8(s);  match_replace(m1 in s -> SENTINEL)   # removes top-8,
    m2 = max8(s);  match_replace(m2 in s -> SENTINEL)   # then ranks 9-16.
    match_replace consumes one occurrence per matched value, scanning from
    index 0 -- identical multiplicity/tie semantics to top_k.  [DVE engine]
    fast path:  mask = relu(-s) (1 at sentinel spots)   [Act]
                out  = mask * x                         [Pool]
    bias path:  out  = (s == SENTINEL) * x  fused       [DVE]

Sharding: x [4,4096,4096] -> tokens [16384, 4096]; 2048 contiguous tokens per
core; bias replicated (only loaded when nonzero).
"""

import sys

for _p in ("/opt/trn_rl_repo", "/opt/pypackages"):
    if _p not in sys.path:
        sys.path.insert(0, _p)

import numpy as np

import concourse.bacc as bacc
import concourse.bass as bass
import concourse.mybir as mybir
import concourse.tile as tile
from concourse.bass_utils import run_bass_kernel_spmd

N_CORES = 8
B, S, H = 4, 4096, 4096
TOKENS = B * S                      # 16384
TOK_PER_CORE = TOKENS // N_CORES    # 2048
BANK = 128
N_BANKS = H // BANK                 # 32
TOPK = 16
ROWS_PER_CORE = TOK_PER_CORE * N_BANKS  # 65536

K_ROWS = 32                         # (token,bank) rows per partition per tile
TILE_F = K_ROWS * BANK              # 4096 fp32 elements of free dim
TILE_ROWS = 128 * K_ROWS            # 4096 rows per tile
N_TILES = ROWS_PER_CORE // TILE_ROWS  # 16

F32 = mybir.dt.float32
FAST_SENTINEL = -1.0                # scores >= 0 on the fast (bias==0) path
SLOW_SENTINEL = -float(2.0 ** 126)  # general path sentinel, matched by equality

_program_cache: dict = {}


def _build_program(with_bias: bool, n_tiles: int = N_TILES) -> bass.Bass:
    nc = bacc.Bacc(
        "TRN2",
        target_bir_lowering=False,
        debug=False,
        enable_asserts=False,
        num_devices=N_CORES,
    )
    x = nc.dram_tensor("x", [TOK_PER_CORE, H], F32, kind="ExternalInput")
    out = nc.dram_tensor("out", [TOK_PER_CORE, H], F32, kind="ExternalOutput")
    bias = (
        nc.dram_tensor("bias", [N_BANKS, BANK], F32, kind="ExternalInput")
        if with_bias
        else None
    )
    sentinel = SLOW_SENTINEL if with_bias else FAST_SENTINEL

    with tile.TileContext(nc) as tc:
        with (
            tc.tile_pool(name="xp", bufs=3) as xp,
            tc.tile_pool(name="sp", bufs=2) as sp,
            tc.tile_pool(name="mp", bufs=6) as mp,
            tc.tile_pool(name="kp", bufs=2) as kp,
            tc.tile_pool(name="op", bufs=2) as op,
            tc.tile_pool(name="bp", bufs=1) as bp,
        ):
            bt = None
            if with_bias:
                # with K_ROWS == 32, row (t, p, k) has bank == k, so the bias
                # tile is partition-independent: bt[p, k*128+e] = bias[k, e]
                bt = bp.tile([128, TILE_F], F32)
                nc.sync.dma_start(
                    bt[:],
                    bass.AP(bias, 0, [[0, 128], [BANK, K_ROWS], [1, BANK]]),
                )

            # Software-pipelined prologue: load + abs of tile t+1 are issued
            # BEFORE tile t's mask/select work, so the Act engine never makes
            # the DVE wait at tile boundaries.  The first tile is loaded in
            # quarters so the DVE can start after ~1/4 of the DMA+abs latency
            # (subtile deps let max8 of slice k wait only on its quarter).
            def load_and_abs(t):
                base = t * 128 * TILE_F
                xt = xp.tile([128, TILE_F], F32)
                st = sp.tile([128, TILE_F], F32)
                n_parts = 4 if t == 0 else 1
                q = TILE_F // n_parts
                for i in range(n_parts):
                    fr = slice(i * q, (i + 1) * q)
                    nc.sync.dma_start(
                        xt[:, fr],
                        bass.AP(x, base + i * q, [[TILE_F, 128], [1, q]]),
                    )
                    # s = |x| on Act (sign-bit clear; verified bit-exact on HW)
                    nc.scalar.activation(
                        st[:, fr], xt[:, fr], mybir.ActivationFunctionType.Abs
                    )
                    if with_bias:
                        nc.gpsimd.tensor_tensor(
                            st[:, fr], st[:, fr], bt[:, fr],
                            op=mybir.AluOpType.add,
                        )
                return xt, st

            pending = load_and_abs(0)
            for t in range(n_tiles):
                base = t * 128 * TILE_F
                xt, st = pending

                m1 = mp.tile([128, 8 * K_ROWS], F32)
                m2 = mp.tile([128, 8 * K_ROWS], F32)
                # Ping-pong st <-> pt so match_replace never writes its own
                # input region (avoids same-bank read/write on the DVE).
                pt = st if with_bias else kp.tile([128, TILE_F], F32)
                for k in range(K_ROWS):
                    nc.vector.max(
                        out=m1[:, k * 8 : (k + 1) * 8],
                        in_=st[:, k * BANK : (k + 1) * BANK],
                    )
                for k in range(K_ROWS):
                    sl = st[:, k * BANK : (k + 1) * BANK]
                    nc.vector.match_replace(
                        out=pt[:, k * BANK : (k + 1) * BANK],
                        in_to_replace=m1[:, k * 8 : (k + 1) * 8],
                        in_values=sl,
                        imm_value=sentinel,
                    )
                for k in range(K_ROWS):
                    nc.vector.max(
                        out=m2[:, k * 8 : (k + 1) * 8],
                        in_=pt[:, k * BANK : (k + 1) * BANK],
                    )
                for k in range(K_ROWS):
                    sl = pt[:, k * BANK : (k + 1) * BANK]
                    nc.vector.match_replace(
                        out=st[:, k * BANK : (k + 1) * BANK],
                        in_to_replace=m2[:, k * 8 : (k + 1) * 8],
                        in_values=sl,
                        imm_value=sentinel,
                    )

                if t + 1 < n_tiles:
                    pending = load_and_abs(t + 1)

                # The last tile's mask/multiply/store runs in quarters: each
                # quarter only depends on its own slices' round-2 results, so
                # the post-DVE tail shrinks to ~1/4 of a tile.
                ot = op.tile([128, TILE_F], F32)
                mt = None if with_bias else kp.tile([128, TILE_F], F32)
                n_parts = 4 if t == n_tiles - 1 else 1
                q = TILE_F // n_parts
                for i in range(n_parts):
                    fr = slice(i * q, (i + 1) * q)
                    if with_bias:
                        # general path: select by exact sentinel equality on DVE
                        nc.vector.scalar_tensor_tensor(
                            ot[:, fr],
                            st[:, fr],
                            sentinel,
                            xt[:, fr],
                            op0=mybir.AluOpType.is_equal,
                            op1=mybir.AluOpType.mult,
                        )
                    else:
                        # fast path: scores >= 0, sentinel -1.  mask = relu(-s)
                        # is exactly 1.0 at the sentinel positions, 0 elsewhere
                        nc.scalar.activation(
                            mt[:, fr],
                            st[:, fr],
                            mybir.ActivationFunctionType.Relu,
                            scale=-1.0,
                        )
                        nc.gpsimd.tensor_tensor(
                            ot[:, fr], mt[:, fr], xt[:, fr],
                            op=mybir.AluOpType.mult,
                        )
                    nc.scalar.dma_start(
                        bass.AP(out, base + i * q, [[TILE_F, 128], [1, q]]),
                        ot[:, fr],
                    )
    nc.compile()
    return nc


def _get_program(with_bias: bool) -> bass.Bass:
    key = ("v3", with_bias)
    if key not in _program_cache:
        _program_cache[key] = _build_program(with_bias)
    return _program_cache[key]


def _run_on_hw(x_flat: np.ndarray, bias_np: np.ndarray, trace: bool = False):
    """x_flat: [TOKENS, H] float32. Returns (out_flat [TOKENS, H], results)."""
    with_bias = bool(np.any(bias_np))
    nc = _get_program(with_bias)
    shards = x_flat.reshape(N_CORES, TOK_PER_CORE, H)
    in_maps = []
    for c in range(N_CORES):
        m = {"x": np.ascontiguousarray(shards[c])}
        if with_bias:
            m["bias"] = np.ascontiguousarray(bias_np)
        in_maps.append(m)
    res = run_bass_kernel_spmd(
        nc, in_maps, core_ids=list(range(N_CORES)), trace=trace
    )
    out_flat = np.concatenate(
        [r["out"].reshape(1, TOK_PER_CORE, H) for r in res.results], axis=0
    ).reshape(TOKENS, H)
    return out_flat, res


def kernel(x, bias, num_assigned_tokens):
    x_np = np.ascontiguousarray(np.asarray(x, dtype=np.float32))
    bias_np = np.asarray(bias, dtype=np.float32)
    nat = np.asarray(num_assigned_tokens, dtype=np.float32)
    assert x_np.shape == (B, S, H), x_np.shape

    out_flat, _ = _run_on_hw(x_np.reshape(TOKENS, H), bias_np)
    return out_flat.reshape(B, S, H), nat
